# revision 36
# baseline (speedup 1.0000x reference)
"""Multi-head attention with exclusive post-processing, sharded over 8 trn2 cores.

Sharding: data-parallel over batch (2) x tensor-parallel over heads (16 -> 4/core).
Each core computes a partial transposed output [D, S] for its batch from its 4
heads; the host sums the 4 partials per batch, transposes back, and adds bo.

Device layouts are feature-major ("T" = [feature, position]) so every matmul
contraction sits on the partition axis:
  QT/KT [256, S]     <- W.T @ x.T  (bf16, head pairs stacked on partitions)
  v^T   [64, S]/head (base partition 0 so DVE ops stay partition-aligned)
  scoresT [keys, q]  <- KT_h slices.T @ QT_h
  P^T = exp(scoresT/8)   (ScalarE, scale folded into the activation)
  Y'[128, q] <- [V_h | ones].T @ P^T : rows 0..63 = unnormalized Y, rows
  64..127 = softmax denominator broadcast across partitions for free.
  Exclusive step in closed form: y_excl = (Y - (Y.v)/(sum v^2 + eps) v)/denom,
  with both reciprocals computed as exp(-ln(x)) on ScalarE (ln and exp share
  one ACT table set; DVE's iterative RECIPROCAL is ~8x slower).
  out^T[D, S] <- Wo_h.T slices @ y_excl (bf16, per-head K=64 contraction).

Phase D is split: D1 keeps PE/ACT dense (only a PSUM->SBUF copy and a Ln per
head leave the kc loop); D2 does the exclusive tail off the PE critical path,
interleaving with the next q-block's D1 and the out-projection.
"""

import os
from contextlib import ExitStack

import ml_dtypes
import numpy as np

import concourse.bass as bass
import concourse.mybir as mybir
import concourse.tile as tile
from concourse import bacc, bass_utils
from concourse.alu_op_type import AluOpType
from concourse.bass_isa import ReduceOp

F32 = mybir.dt.float32
F32R = mybir.dt.float32r
BF16 = mybir.dt.bfloat16
AF = mybir.ActivationFunctionType

B, S_FULL, D_FULL, H_FULL = 2, 2048, 1024, 16
HD = 64
N_CORES = 8
HEADS_PER_CORE = H_FULL * B // N_CORES  # 4


def build_nc(S=S_FULL, D=D_FULL, HL=HEADS_PER_CORE, use_bias=False):
    """Build the per-core Bass kernel. Returns a finalized Bacc object."""
    P = 128
    nH = HL * HD          # local fused head dim (256)
    KC = D // P           # x contraction chunks (8)
    NKc = S // P          # key chunks (16)
    QB = min(1024, S)     # q block (PSUM-sized)
    NQ = S // QB
    MT = nH // P          # feature M-tiles for QT/KT (2)
    DM = D // P           # out-proj M-tiles (8)
    NS = min(512, QB)     # matmul moving-dim chunk

    assert S % P == 0 and D % P == 0 and nH % P == 0 and QB % NS == 0

    _ensure_act_root()
    nc = bacc.Bacc(None, target_bir_lowering=False)

    xT_d = nc.dram_tensor("xT", [D, S], BF16, kind="ExternalInput")
    wq_d = nc.dram_tensor("wq", [D, nH], BF16, kind="ExternalInput")
    wk_d = nc.dram_tensor("wk", [D, nH], BF16, kind="ExternalInput")
    wv_d = nc.dram_tensor("wv", [D, nH], BF16, kind="ExternalInput")
    wo_d = nc.dram_tensor("wo", [nH, D], BF16, kind="ExternalInput")
    if use_bias:
        bq_d = nc.dram_tensor("bq", [1, nH], F32, kind="ExternalInput")
        bk_d = nc.dram_tensor("bk", [1, nH], F32, kind="ExternalInput")
        bv_d = nc.dram_tensor("bv", [1, nH], F32, kind="ExternalInput")
    outT_d = nc.dram_tensor("outT", [D, S], F32, kind="ExternalOutput")

    with tile.TileContext(nc) as tc, ExitStack() as ctx:
        consts = ctx.enter_context(tc.tile_pool(name="consts", bufs=1))
        psA = ctx.enter_context(tc.tile_pool(name="psA", bufs=2, space="PSUM"))
        psB = ctx.enter_context(tc.tile_pool(name="psB", bufs=2, space="PSUM"))
        pP = ctx.enter_context(tc.tile_pool(name="pP", bufs=3))
        stk = ctx.enter_context(tc.tile_pool(name="stk", bufs=2))
        bcs = ctx.enter_context(tc.tile_pool(name="bcs", bufs=2))
        bcs2 = ctx.enter_context(tc.tile_pool(name="bcs2", bufs=2))
        tps = ctx.enter_context(tc.tile_pool(name="tps", bufs=2))
        tps2 = ctx.enter_context(tc.tile_pool(name="tps2", bufs=2))
        ysbp = ctx.enter_context(tc.tile_pool(name="ysbp", bufs=6))
        lndp = ctx.enter_context(tc.tile_pool(name="lndp", bufs=5))

        # ---- ACT table preload: dummy exp+ln force the (single) table-set
        # load at kernel start, not as a 2.7us PE-stalling hiccup at the
        # start of the attention phase (which re-throttles the PE clock).
        smallc = consts.tile([P, 33], F32, tag="smallc")
        warm = smallc[0:1, 1:33]
        nc.vector.memset(warm, 1.0)
        nc.scalar.activation(out=warm, in_=warm, func=AF.Exp)
        nc.scalar.activation(out=warm, in_=warm, func=AF.Ln)

        # ---- input staging ----
        # Q/K weight chunks first (small), so the first projection matmuls
        # only wait for their own xT chunk, not the whole input stream.
        def load_w(dram):
            tiles = []
            for kc in range(KC):
                t = consts.tile([P, nH], BF16, tag=f"w{dram.name}{kc}")
                nc.sync.dma_start(out=t, in_=dram.ap()[kc * P:(kc + 1) * P, :])
                tiles.append(t)
            return tiles

        wq_sb, wk_sb = load_w(wq_d), load_w(wk_d)

        xT_sb = []
        for kc in range(KC):
            t = consts.tile([P, S], BF16, tag=f"xT{kc}")
            nc.sync.dma_start(out=t, in_=xT_d.ap()[kc * P:(kc + 1) * P, :])
            xT_sb.append(t)

        wv_sb = load_w(wv_d)

        wo_bf = []
        for h in range(HL):
            wbf = consts.tile([HD, D], BF16, tag=f"wobf_{h}", name=f"wobf_{h}")
            nc.sync.dma_start(out=wbf, in_=wo_d.ap()[h * HD:(h + 1) * HD, :])
            wo_bf.append(wbf)

        if use_bias:
            ones_row = consts.tile([1, max(S, P)], F32, tag="ones_row")
            nc.vector.memset(ones_row, 1.0)
            b_sb = {}
            for name, dram in (("q", bq_d), ("k", bk_d), ("v", bv_d)):
                t = consts.tile([1, nH], F32, tag=f"b{name}")
                nc.sync.dma_start(out=t, in_=dram.ap())
                b_sb[name] = t

        # eps vector for the ln(sum v^2 + eps) bias
        epsv = smallc[:, 0:1]
        nc.vector.memset(epsv, 1e-12)
        # ones64x64: all-ones [64,64] -> column-sum matmuls produce the result
        # broadcast across all 64 output partitions for free
        ones64x64 = consts.tile([HD, HD], BF16, tag="ones64x64")
        nc.vector.memset(ones64x64, 1.0)

        # ---- phase B: feature-major projections QT/KT [nH, S] (bf16, head pairs) ----
        QT = [consts.tile([P, S], BF16, tag=f"QT{t_i}", name=f"QT{t_i}") for t_i in range(MT)]
        KT = [consts.tile([P, S], BF16, tag=f"KT{t_i}", name=f"KT{t_i}") for t_i in range(MT)]

        def emit_qk(mt):
            for w_sb, dst, bias_key in ((wq_sb, QT, "q"), (wk_sb, KT, "k")):
                for qb in range(NQ):
                    ps = psA.tile([P, QB], F32, tag="ps", name="ps_qk")
                    if use_bias:
                        for ns in range(0, QB, NS):
                            nc.tensor.matmul(
                                ps[:, ns:ns + NS],
                                lhsT=b_sb[bias_key][:, mt * P:(mt + 1) * P].bitcast(F32R),
                                rhs=ones_row[:, :NS].bitcast(F32R),
                                start=True, stop=False)
                    for kc in range(KC):
                        for ns in range(0, QB, NS):
                            nc.tensor.matmul(
                                ps[:, ns:ns + NS],
                                lhsT=w_sb[kc][:, mt * P:(mt + 1) * P],
                                rhs=xT_sb[kc][:, qb * QB + ns:qb * QB + ns + NS],
                                start=(kc == 0 and not use_bias), stop=(kc == KC - 1))
                    nc.vector.tensor_copy(out=dst[mt][:, qb * QB:(qb + 1) * QB], in_=ps)

        # ---- phase B2: per-head v^T [64, S] at base partition 0 (DVE alignment) ----
        VTh = [consts.tile([HD, S], BF16, tag=f"VTh{h}", name=f"VTh{h}") for h in range(HL)]

        def emit_vth(h):
            for qb in range(NQ):
                ps = psA.tile([HD, QB], F32, tag="ps", name="ps_vth")
                if use_bias:
                    for ns in range(0, QB, NS):
                        nc.tensor.matmul(
                            ps[:, ns:ns + NS],
                            lhsT=b_sb["v"][:, h * HD:(h + 1) * HD].bitcast(F32R),
                            rhs=ones_row[:, :NS].bitcast(F32R),
                            start=True, stop=False)
                for kc in range(KC):
                    for ns in range(0, QB, NS):
                        nc.tensor.matmul(
                            ps[:, ns:ns + NS],
                            lhsT=wv_sb[kc][:, h * HD:(h + 1) * HD],
                            rhs=xT_sb[kc][:, qb * QB + ns:qb * QB + ns + NS],
                            start=(kc == 0 and not use_bias), stop=(kc == KC - 1))
                nc.vector.tensor_copy(out=VTh[h][:, qb * QB:(qb + 1) * QB], in_=ps)

        # ---- phase C: position-major V with a 64-wide ones block ----
        vprime = consts.tile([P, NKc, HL, 2 * HD], BF16, tag="vprime")

        def emit_vprime():
            nc.vector.memset(vprime[:, :, :, HD:2 * HD], 1.0)
            for qt in range(NKc):
                ps = psA.tile([P, nH], F32, tag="ps", name="ps_vp")
                if use_bias:
                    nc.tensor.matmul(
                        ps, lhsT=ones_row[:, 0:P].bitcast(F32R),
                        rhs=b_sb["v"].bitcast(F32R), start=True, stop=False)
                for kc in range(KC):
                    nc.tensor.matmul(
                        ps,
                        lhsT=xT_sb[kc][:, qt * P:(qt + 1) * P],
                        rhs=wv_sb[kc],
                        start=(kc == 0 and not use_bias), stop=(kc == KC - 1))
                nc.vector.tensor_copy(
                    out=vprime[:, qt, :, 0:HD],
                    in_=ps.rearrange("p (h d) -> p h d", h=HL))

        def head_slice(tiles, h):
            return tiles[h // 2][64 * (h % 2):64 * (h % 2) + 64, :]

        y_excl = [consts.tile([HD, S], BF16, tag=f"yx{h}", name=f"yx{h}") for h in range(HL)]

        def emit_d1(qb, h):
            q0 = qb * QB
            KTh, QTh = (head_slice(t, h) for t in (KT, QT))
            yp = psB.tile([P, QB], F32, tag="yp", name=f"yp{h}")
            for kc in range(NKc):
                sc = psA.tile([P, QB], F32, tag="ps", name=f"sc{h}")
                for ns in range(0, QB, NS):
                    nc.tensor.matmul(
                        sc[:, ns:ns + NS],
                        lhsT=KTh[:, kc * P:(kc + 1) * P],
                        rhs=QTh[:, q0 + ns:q0 + ns + NS],
                        start=True, stop=True)
                pT = pP.tile([P, QB], BF16, tag="pt", name=f"pt{h}")
                nc.scalar.activation(out=pT, in_=sc, func=AF.Exp, scale=0.125)
                for ns in range(0, QB, NS):
                    nc.tensor.matmul(
                        yp[:, ns:ns + NS],
                        lhsT=vprime[:, kc, h, :],
                        rhs=pT[:, ns:ns + NS],
                        start=(kc == 0), stop=(kc == NKc - 1))
            ysb = ysbp.tile([HD, QB], BF16, tag="ysb", name=f"ysb{h}")
            nc.vector.tensor_copy(out=ysb, in_=yp[0:HD, :])
            lnden = lndp.tile([HD, QB], F32, tag="lnd", name=f"lnden{h}")
            nc.scalar.activation(out=lnden, in_=yp[HD:2 * HD, :], func=AF.Ln)
            return ysb, lnden

        def emit_pre(qb, h):
            """1/(sum v^2 + eps), broadcast -- independent of the attention
            output, so it runs alongside D1 and keeps D2's chain short."""
            q0 = qb * QB
            vth = VTh[h]
            vsq = stk.tile([HD, QB], BF16, tag="vsq")
            nc.vector.tensor_mul(vsq, vth[:, q0:q0 + QB], vth[:, q0:q0 + QB])
            d2B = psB.tile([HD, QB], F32, tag="yp", name="d2B")
            for ns in range(0, QB, NS):
                nc.tensor.matmul(d2B[:, ns:ns + NS], lhsT=ones64x64,
                                 rhs=vsq[:, ns:ns + NS], start=True, stop=True)
            lns = bcs2.tile([HD, QB], F32, tag="lns")
            nc.scalar.activation(out=lns, in_=d2B, func=AF.Ln, bias=epsv[0:HD, :])
            r2B = bcs2.tile([HD, QB], BF16, tag="r2b")
            nc.scalar.activation(out=r2B, in_=lns, func=AF.Exp, scale=-1.0)
            return r2B

        def emit_d2(qb, h, ysb, lnden, r2B):
            q0 = qb * QB
            vth = VTh[h]
            t_yv = stk.tile([HD, QB], BF16, tag="t_yv")
            nc.vector.tensor_mul(t_yv, ysb, vth[:, q0:q0 + QB])
            d1B = psB.tile([HD, QB], F32, tag="yp", name="d1B")
            for ns in range(0, QB, NS):
                nc.tensor.matmul(d1B[:, ns:ns + NS], lhsT=ones64x64,
                                 rhs=t_yv[:, ns:ns + NS], start=True, stop=True)

            betaB = bcs.tile([HD, QB], F32, tag="bet")
            nc.scalar.activation(out=betaB, in_=lnden, func=AF.Exp, scale=-1.0)

            aB = stk.tile([HD, QB], BF16, tag="ab")
            nc.vector.tensor_mul(aB, d1B[0:HD, :], r2B)
            t2 = tps2.tile([HD, QB], BF16, tag="t2")
            nc.vector.tensor_mul(t2, vth[:, q0:q0 + QB], aB)
            u = tps.tile([HD, QB], BF16, tag="t1")
            nc.vector.tensor_sub(u, ysb, t2)
            nc.vector.tensor_mul(y_excl[h][:, q0:q0 + QB], u, betaB)

        def emit_e(qb, mt0=0, mt1=None):
            for mt in range(mt0, DM if mt1 is None else mt1):
                ps = psA.tile([P, QB], F32, tag="ps", name="ps_e")
                for h in range(HL):
                    lw = wo_bf[h][:, mt * P:(mt + 1) * P]
                    for ns in range(0, QB, NS):
                        nc.tensor.matmul(
                            ps[:, ns:ns + NS],
                            lhsT=lw,
                            rhs=y_excl[h][:, qb * QB + ns:qb * QB + ns + NS],
                            start=(h == 0), stop=(h == HL - 1))
                ostg = pP.tile([P, QB], F32, tag="ostg")
                nc.any.tensor_copy(out=ostg, in_=ps)
                nc.sync.dma_start(
                    out=outT_d.ap()[mt * P:(mt + 1) * P, qb * QB:(qb + 1) * QB],
                    in_=ostg)

        # ---- emission order: get the ACT-bound attention started early, then
        # feed the PE the remaining projection work to fill its dependency
        # cracks, so the PE never idles long enough to re-throttle. ----
        emit_qk(0)           # Q,K for heads 0,1
        emit_vprime()        # V' (needed by attn@V)
        saved = {}

        def d2_block(qb):
            # the 1/(sum v^2+eps) chains first: independent of the attention
            # output, they overlap the still-running D1s of the next q-block
            r2Bs = [emit_pre(qb, h) for h in range(HL)]
            for h in range(HL):
                emit_d2(qb, h, *saved[(qb, h)], r2Bs[h])

        saved[(0, 0)] = emit_d1(0, 0)
        emit_qk(1)           # Q,K heads 2,3 -- PE filler during D1 ACT stretches
        saved[(0, 1)] = emit_d1(0, 1)
        for h in range(HL):
            emit_vth(h)      # v^T per head -- more PE filler
        saved[(0, 2)] = emit_d1(0, 2)
        saved[(0, 3)] = emit_d1(0, 3)
        if NQ > 1:
            for qb in range(1, NQ):
                # two D1s of the next q-block first: their scores fill the PE
                # while the previous block's exclusive tail runs on ACT/DVE
                saved[(qb, 0)] = emit_d1(qb, 0)
                saved[(qb, 1)] = emit_d1(qb, 1)
                d2_block(qb - 1)
                for h in range(2, HL):
                    saved[(qb, h)] = emit_d1(qb, h)
                emit_e(qb - 1)
            d2_block(NQ - 1)
            emit_e(NQ - 1)
        else:
            d2_block(0)
            emit_e(0)

    nc.finalize()
    return nc


def shard_inputs(x, Wq, bq, Wk, bk, Wv, bv, Wo, bo, n_cores=N_CORES):
    """Full inputs -> per-core input maps (host-side transpose/slice/reshape)."""
    H = Wq.shape[1]
    cores_per_batch = n_cores // x.shape[0]
    hl = H // cores_per_batch
    in_maps = []
    for c in range(n_cores):
        b = c // cores_per_batch
        h0 = (c % cores_per_batch) * hl
        bf = ml_dtypes.bfloat16
        m = {
            "xT": np.ascontiguousarray(x[b].T).astype(bf),
            "wq": np.ascontiguousarray(Wq[:, h0:h0 + hl, :].reshape(Wq.shape[0], -1)).astype(bf),
            "wk": np.ascontiguousarray(Wk[:, h0:h0 + hl, :].reshape(Wk.shape[0], -1)).astype(bf),
            "wv": np.ascontiguousarray(Wv[:, h0:h0 + hl, :].reshape(Wv.shape[0], -1)).astype(bf),
            "wo": np.ascontiguousarray(Wo[h0:h0 + hl].reshape(-1, Wo.shape[2])).astype(bf),
        }
        if _use_bias(bq, bk, bv):
            m["bq"] = np.ascontiguousarray(bq[h0:h0 + hl].reshape(1, -1)).astype(np.float32)
            m["bk"] = np.ascontiguousarray(bk[h0:h0 + hl].reshape(1, -1)).astype(np.float32)
            m["bv"] = np.ascontiguousarray(bv[h0:h0 + hl].reshape(1, -1)).astype(np.float32)
        in_maps.append(m)
    return in_maps


def _use_bias(bq, bk, bv):
    return bool(np.any(bq) or np.any(bk) or np.any(bv))


_ACT_ROOT_READY = False


def _ensure_act_root():
    """Point walrus at an act-table root whose only set is
    natural_log_exp_and_others, so exp and ln share one ACT table set and the
    kernel never pays mid-stream ACT_TABLE_LOADs (which stall the PE long
    enough to re-throttle its clock)."""
    global _ACT_ROOT_READY
    if _ACT_ROOT_READY or os.environ.get("BASS_ACT_ROOT_JSON_PATH"):
        _ACT_ROOT_READY = True
        return
    import json
    import tempfile
    from neuronxcc.driver.Job import Job
    from neuronxcc.driver.jobs.support.FindActInfo import findActInfoFile

    orig = findActInfoFile(Job.getPackageDir(), "gen3")
    with open(orig) as f:
        info = json.load(f)
    keep = [e for e in info["act_func_sets"]
            if e["name"] == "natural_log_exp_and_others"]
    if not keep:  # unexpected layout -- fall back to stock tables
        _ACT_ROOT_READY = True
        return
    root = tempfile.mkdtemp(prefix="act_root_")
    src_dir = os.path.dirname(orig)
    for fn in os.listdir(src_dir):
        if fn != "act_info.json":
            os.symlink(os.path.join(src_dir, fn), os.path.join(root, fn))
    info["act_func_sets"] = keep
    with open(os.path.join(root, "act_info.json"), "w") as f:
        json.dump(info, f)
    os.environ["BASS_ACT_ROOT_JSON_PATH"] = os.path.join(root, "act_info.json")

    # Bacc preplaces InstLoadActFuncSet using concourse.hw_specs tables (it
    # reads the stock act_info directly); keep its set-id numbering in sync
    # with the custom single-set root.
    import concourse.hw_specs as hw_specs
    import concourse.bacc as bacc_mod
    _orig_tables = hw_specs.get_activation_tables

    def _single_set_tables(module_arch):
        tables = _orig_tables(module_arch)
        if "natural_log_exp_and_others" in tables:
            return {"natural_log_exp_and_others": tables["natural_log_exp_and_others"]}
        return tables

    hw_specs.get_activation_tables = _single_set_tables
    bacc_mod.get_activation_tables = _single_set_tables
    _ACT_ROOT_READY = True


_NC_CACHE = {}


def _get_nc(use_bias):
    if use_bias not in _NC_CACHE:
        _NC_CACHE[use_bias] = build_nc(use_bias=use_bias)
    return _NC_CACHE[use_bias]


def run_sharded(inputs, trace=False, trace_cores=None):
    """Run the SPMD kernel; returns (full_output, BassKernelResults)."""
    x, bo = inputs["x"], inputs["bo"]
    use_bias = _use_bias(inputs["bq"], inputs["bk"], inputs["bv"])
    _ensure_act_root()
    nc = _get_nc(use_bias)
    in_maps = shard_inputs(**inputs)
    res = bass_utils.run_bass_kernel_spmd(
        nc, in_maps, core_ids=list(range(N_CORES)),
        trace=trace, trace_cores=trace_cores)
    cores_per_batch = N_CORES // x.shape[0]
    out = np.empty_like(x)
    for b in range(x.shape[0]):
        acc = np.zeros((x.shape[2], x.shape[1]), np.float32)
        for c in range(b * cores_per_batch, (b + 1) * cores_per_batch):
            acc += res.results[c]["outT"]
        out[b] = acc.T + bo[None, :]
    return out, res


def kernel(**inputs):
    out, _ = run_sharded(inputs)
    return out
